# revision 8
# baseline (speedup 1.0000x reference)
"""BitNet dense layer on 8 Trainium2 NeuronCores.

reference math:
    row_scale = clip(mean(|W|, axis=1), 1e-8)        # [out]
    out = (x @ sign(W).T) * row_scale * scale_param  # [B,S,out]

Strategy (data-parallel over the 8192 tokens):
  * Host folds row_scale * scale_param into the binarized weight:
        Wf = sign(W) * comb[:, None]   -> bf16, exactly +-comb[o] per row
    so the device kernel is a single plain matmul.
  * Host pre-transposes both operands so the device streams natural-layout
    [K, *] tiles (contraction dim on partitions) with zero on-chip transposes:
        xT [4096, 8192] bf16 (sharded 1024 tokens/core), wT [4096, 4096] bf16.
  * Each core computes out_c[1024, 4096] f32 = xT_c.T @ wT via the production
    tile matmul kernel; host concatenates the 8 shards.
"""

import numpy as np
import ml_dtypes

B, S, D_IN, D_OUT = 4, 2048, 4096, 4096
N_CORES = 8
M_TOT = B * S
M_LOC = M_TOT // N_CORES

_prog = None
last_results = None  # BassKernelResults of the most recent run (for test harness)
TRACE = False  # set True by the dev test harness (needs NTFF shims) to profile


def _build_program():
    import concourse.tile as tile
    from concourse import bacc, mybir
    from concourse.kernels.tile_matmul import matmul_tile_kernel

    nc = bacc.Bacc(
        "TRN2", target_bir_lowering=False, debug=False, num_devices=N_CORES
    )
    xT = nc.dram_tensor(
        "xT", [D_IN, M_LOC], mybir.dt.bfloat16, kind="ExternalInput"
    ).ap()
    wT = nc.dram_tensor(
        "wT", [D_IN, D_OUT], mybir.dt.bfloat16, kind="ExternalInput"
    ).ap()
    out = nc.dram_tensor(
        "out", [M_LOC, D_OUT], mybir.dt.float32, kind="ExternalOutput"
    ).ap()
    with tile.TileContext(nc) as tc:
        matmul_tile_kernel(
            tc,
            kxm_ap=xT,
            kxn_ap=wT,
            mxn_ap=out,
        )
    nc.compile()
    return nc


def kernel(input, weight, scale_param):
    global _prog, last_results
    from concourse.bass_utils import run_bass_kernel_spmd

    x = np.asarray(input, dtype=np.float32).reshape(M_TOT, D_IN)
    W = np.asarray(weight, dtype=np.float32)
    sp = np.asarray(scale_param, dtype=np.float32)

    comb = np.clip(np.abs(W).mean(axis=1, dtype=np.float32), 1e-8, None) * sp
    wT = (np.sign(W) * comb[:, None].astype(np.float32)).T.astype(
        ml_dtypes.bfloat16, order="C"
    )
    xT = x.T.astype(ml_dtypes.bfloat16, order="C")

    if _prog is None:
        _prog = _build_program()

    in_maps = [
        {
            "xT": np.ascontiguousarray(xT[:, c * M_LOC : (c + 1) * M_LOC]),
            "wT": wT,
        }
        for c in range(N_CORES)
    ]
    last_results = run_bass_kernel_spmd(
        _prog, in_maps, list(range(N_CORES)), trace=TRACE
    )
    out = np.concatenate(
        [last_results.results[c]["out"] for c in range(N_CORES)], axis=0
    )
    return np.nan_to_num(
        out.reshape(B, S, D_OUT), nan=0.0, posinf=1e6, neginf=-1e6
    )


# revision 9
# speedup vs baseline: 1.0009x; 1.0009x over previous
"""BitNet dense layer on 8 Trainium2 NeuronCores.

reference math:
    row_scale = clip(mean(|W|, axis=1), 1e-8)        # [out]
    out = (x @ sign(W).T) * row_scale * scale_param  # [B,S,out]

Strategy (data-parallel over the 8192 tokens):
  * Host folds row_scale * scale_param into the binarized weight:
        Wf = sign(W) * comb[:, None]   -> bf16, exactly +-comb[o] per row
    so the device kernel is a single plain matmul.
  * Host pre-transposes both operands so the device streams natural-layout
    [K, *] tiles (contraction dim on partitions) with zero on-chip transposes:
        xT [4096, 8192] bf16 (sharded 1024 tokens/core), wT [4096, 4096] bf16.
  * Each core computes out_c[1024, 4096] f32 = xT_c.T @ wT via the production
    tile matmul kernel; host concatenates the 8 shards.
"""

import numpy as np
import ml_dtypes

B, S, D_IN, D_OUT = 4, 2048, 4096, 4096
N_CORES = 8
M_TOT = B * S
M_LOC = M_TOT // N_CORES

_prog = None
last_results = None  # BassKernelResults of the most recent run (for test harness)
TRACE = False  # set True by the dev test harness (needs NTFF shims) to profile


def _build_program():
    import concourse.tile as tile
    from concourse import bacc, mybir
    from concourse.kernels.tile_matmul import matmul_tile_kernel

    nc = bacc.Bacc(
        "TRN2", target_bir_lowering=False, debug=False, num_devices=N_CORES
    )
    xT = nc.dram_tensor(
        "xT", [D_IN, M_LOC], mybir.dt.bfloat16, kind="ExternalInput"
    ).ap()
    wT = nc.dram_tensor(
        "wT", [D_IN, D_OUT], mybir.dt.bfloat16, kind="ExternalInput"
    ).ap()
    out = nc.dram_tensor(
        "out", [M_LOC, D_OUT], mybir.dt.float32, kind="ExternalOutput"
    ).ap()
    with tile.TileContext(nc) as tc:
        # PE warmup: ~20 dummy matmuls run while the first real tiles DMA in,
        # releasing the HAM clock gate (1.2 -> 2.4 GHz takes ~3.4us of PE
        # activity) so the real matmul stream starts at full clock.
        with (
            tc.tile_pool(name="warm", bufs=1) as warm,
            tc.tile_pool(name="warm_psum", bufs=1, space="PSUM") as warm_psum,
        ):
            wa = warm.tile([128, 128], mybir.dt.bfloat16)
            wb = warm.tile([128, 512], mybir.dt.bfloat16)
            nc.any.memset(wa[:], 0.0)
            nc.any.memset(wb[:], 0.0)
            ps = warm_psum.tile([128, 512], mybir.dt.float32)
            for i in range(20):
                nc.tensor.matmul(ps[:], wa[:], wb[:], start=(i == 0), stop=(i == 19))
        matmul_tile_kernel(
            tc,
            kxm_ap=xT,
            kxn_ap=wT,
            mxn_ap=out,
        )
    nc.compile()
    return nc


def kernel(input, weight, scale_param):
    global _prog, last_results
    from concourse.bass_utils import run_bass_kernel_spmd

    x = np.asarray(input, dtype=np.float32).reshape(M_TOT, D_IN)
    W = np.asarray(weight, dtype=np.float32)
    sp = np.asarray(scale_param, dtype=np.float32)

    comb = np.clip(np.abs(W).mean(axis=1, dtype=np.float32), 1e-8, None) * sp
    wT = (np.sign(W) * comb[:, None].astype(np.float32)).T.astype(
        ml_dtypes.bfloat16, order="C"
    )
    xT = x.T.astype(ml_dtypes.bfloat16, order="C")

    if _prog is None:
        _prog = _build_program()

    in_maps = [
        {
            "xT": np.ascontiguousarray(xT[:, c * M_LOC : (c + 1) * M_LOC]),
            "wT": wT,
        }
        for c in range(N_CORES)
    ]
    last_results = run_bass_kernel_spmd(
        _prog, in_maps, list(range(N_CORES)), trace=TRACE
    )
    out = np.concatenate(
        [last_results.results[c]["out"] for c in range(N_CORES)], axis=0
    )
    return np.nan_to_num(
        out.reshape(B, S, D_OUT), nan=0.0, posinf=1e6, neginf=-1e6
    )


# revision 15
# speedup vs baseline: 1.0048x; 1.0039x over previous
"""BitNet dense layer on 8 Trainium2 NeuronCores.

reference math:
    row_scale = clip(mean(|W|, axis=1), 1e-8)        # [out]
    out = (x @ sign(W).T) * row_scale * scale_param  # [B,S,out]

Strategy (data-parallel over the 8192 tokens):
  * Host folds row_scale * scale_param into the binarized weight:
        Wf = sign(W) * comb[:, None]   -> bf16, exactly +-comb[o] per row
    so the device kernel is a single plain matmul.
  * Host pre-transposes both operands so the device streams natural-layout
    [K, *] tiles (contraction dim on partitions) with zero on-chip transposes:
        xT [4096, 8192] bf16 (sharded 1024 tokens/core), wT [4096, 4096] bf16.
  * Each core computes out_c[1024, 4096] f32 = xT_c.T @ wT via the production
    tile matmul kernel; host concatenates the 8 shards.
"""

from contextlib import ExitStack

import numpy as np
import ml_dtypes

B, S, D_IN, D_OUT = 4, 2048, 4096, 4096
N_CORES = 8
M_TOT = B * S
M_LOC = M_TOT // N_CORES

_prog = None
last_results = None  # BassKernelResults of the most recent run (for test harness)
TRACE = False  # set True by the dev test harness (needs NTFF shims) to profile


def _build_program():
    import concourse.tile as tile
    from concourse import bacc, mybir
    from concourse.bass import ds, ts
    from concourse.kernels.tile_matmul import (
        ShapeInfo,
        composable_matmul_tile_kernel,
    )

    P = 128
    K_SUB = 4  # K_TILE=512 -> 4 subtiles of 128
    M_TILE, N_TILE = 512, 512

    nc = bacc.Bacc(
        "TRN2", target_bir_lowering=False, debug=False, num_devices=N_CORES
    )
    xT = nc.dram_tensor(
        "xT", [D_IN, M_LOC], mybir.dt.bfloat16, kind="ExternalInput"
    ).ap()
    wT = nc.dram_tensor(
        "wT", [D_IN, D_OUT], mybir.dt.bfloat16, kind="ExternalInput"
    ).ap()
    out = nc.dram_tensor(
        "out", [M_LOC, D_OUT], mybir.dt.float32, kind="ExternalOutput"
    ).ap()
    xT3 = xT.rearrange("(po pi) f -> pi po f", pi=P)  # [128, 32, M_LOC]
    wT3 = wT.rearrange("(po pi) f -> pi po f", pi=P)  # [128, 32, D_OUT]
    out3 = out.rearrange("(po pi) f -> pi po f", pi=P)  # [128, 8, D_OUT]
    M_TILES = M_LOC // M_TILE

    with tile.TileContext(nc) as tc:
        # PE warmup: ~20 dummy matmuls run while the first real tiles DMA in,
        # releasing the HAM clock gate (1.2 -> 2.4 GHz takes ~3.4us of PE
        # activity) so the real matmul stream starts at full clock.
        with (
            tc.tile_pool(name="warm", bufs=1) as warm,
            tc.tile_pool(name="warm_psum", bufs=1, space="PSUM") as warm_psum,
        ):
            wa = warm.tile([128, 128], mybir.dt.bfloat16)
            wb = warm.tile([128, 512], mybir.dt.bfloat16)
            nc.any.memset(wa[:], 0.0)
            nc.any.memset(wb[:], 0.0)
            ps = warm_psum.tile([128, 512], mybir.dt.float32)
            for i in range(20):
                nc.tensor.matmul(ps[:], wa[:], wb[:], start=(i == 0), stop=(i == 19))

        tc.swap_default_side()
        num_bufs = D_IN // P // K_SUB + 1  # K_TILES + 1
        stack = ExitStack()
        kxm_pool = stack.enter_context(tc.tile_pool(name="kxm_pool", bufs=num_bufs))
        kxn_pool = stack.enter_context(tc.tile_pool(name="kxn_pool", bufs=num_bufs))

        # Pre-issue subtile 0 of the first kxn tile so the two DMAs gating the
        # first matmul (kxm0.0, kxn0.0) occupy the first issue slots; deps are
        # region-granular so matmuls on subtile j wait only on sub-DMA j.
        first_kxn = kxn_pool.tile([P, K_SUB, N_TILE], mybir.dt.bfloat16, tag="kxn")
        nc.sync.dma_start(first_kxn[:, 0, :], wT3[:, 0, ds(0, N_TILE)])
        state = {"first_kxn": first_kxn}

        def kxm_producer(nc_, md):
            t = kxm_pool.tile(
                [P, md.k_subtiles, md.m_tile], mybir.dt.bfloat16, tag="kxm"
            )
            src = xT3[
                :,
                ts(md.k_tile_idx, md.k_subtiles),
                ds(md.m_tile_idx * md.m_tile, md.m_tile),
            ]
            if md.k_tile_idx == 0 and md.m_tile_idx == 0:
                # Head: split so the first matmul starts after 128KiB, not 512KiB.
                for j in range(md.k_subtiles):
                    nc.sync.dma_start(t[:, j, :], src[:, j, :])
            else:
                nc.sync.dma_start(t[:], src)
            return t[:]

        def kxn_producer(nc_, md):
            src = wT3[
                :,
                ts(md.k_tile_idx, md.k_subtiles),
                ds(md.n_tile_idx * md.n_tile, md.n_tile),
            ]
            if (
                md.k_tile_idx == 0
                and md.n_tile_idx == 0
                and state.get("first_kxn") is not None
            ):
                t = state.pop("first_kxn")
                for j in range(1, md.k_subtiles):
                    nc.sync.dma_start(t[:, j, :], src[:, j, :])
                return t[:]
            t = kxn_pool.tile(
                [P, md.k_subtiles, md.n_tile], mybir.dt.bfloat16, tag="kxn"
            )
            nc.sync.dma_start(t[:], src)
            return t[:]

        def mxn_consumer(nc_, sbuf, md):
            dst = out3[
                :,
                ts(md.m_tile_idx, md.m_subtiles),
                ds(md.n_tile_idx * md.n_tile, md.n_tile),
            ]
            if md.m_tile_idx == M_TILES - 1 and md.n_tile_idx == 0:
                # Tail (last tile in snake order): split per m-subtile so each
                # chunk DMAs out right after its PSUM eviction.
                for j in range(md.m_subtiles):
                    nc.sync.dma_start(dst[:, j, :], sbuf[:, j, :])
            else:
                nc.sync.dma_start(dst, sbuf[:, :, : md.n_tile])

        composable_matmul_tile_kernel(
            tc=tc,
            kxm_shape=ShapeInfo(pdims=((P, D_IN // P),), fdims=(M_LOC,)),
            kxn_shape=ShapeInfo(pdims=((P, D_IN // P),), fdims=(D_OUT,)),
            output_type=mybir.dt.float32,
            kxm_producer=kxm_producer,
            kxn_producer=kxn_producer,
            mxn_consumer=mxn_consumer,
        )
        stack.close()
    nc.compile()
    return nc


def kernel(input, weight, scale_param):
    global _prog, last_results
    from concourse.bass_utils import run_bass_kernel_spmd

    x = np.asarray(input, dtype=np.float32).reshape(M_TOT, D_IN)
    W = np.asarray(weight, dtype=np.float32)
    sp = np.asarray(scale_param, dtype=np.float32)

    comb = np.clip(np.abs(W).mean(axis=1, dtype=np.float32), 1e-8, None) * sp
    wT = (np.sign(W) * comb[:, None].astype(np.float32)).T.astype(
        ml_dtypes.bfloat16, order="C"
    )
    xT = x.T.astype(ml_dtypes.bfloat16, order="C")

    if _prog is None:
        _prog = _build_program()

    in_maps = [
        {
            "xT": np.ascontiguousarray(xT[:, c * M_LOC : (c + 1) * M_LOC]),
            "wT": wT,
        }
        for c in range(N_CORES)
    ]
    last_results = run_bass_kernel_spmd(
        _prog, in_maps, list(range(N_CORES)), trace=TRACE
    )
    out = np.concatenate(
        [last_results.results[c]["out"] for c in range(N_CORES)], axis=0
    )
    return np.nan_to_num(
        out.reshape(B, S, D_OUT), nan=0.0, posinf=1e6, neginf=-1e6
    )
